# revision 2
# baseline (speedup 1.0000x reference)
"""GraphSAGE GNN layer on 8 trn2 cores — v1.5 (safe fallback).

Same compute pipeline as v2 (bf16 table, DVE tree reduce, PE transpose,
bf16 matmul) but gathers with the HW-supported walrus indirect DMA:
one [128,1]-offset instruction per (tile, slot) = 176 per core, each
gathering 128 rows of 512B. Q7 SWDGE-bound but ~2x the f32 baseline.
"""
import numpy as np

N_CORES = 8
NUM_NODES = 1_000_000
F = 256
E = 256
B = 16384
NSAMP = 10
SLOTS = 1 + NSAMP
P = 128
B_LOCAL = B // N_CORES          # 2048
TILES = B_LOCAL // P            # 16
GROUP = 4

_cache = {}


def _build():
    import concourse.bass as bass
    import concourse.bacc as bacc
    import concourse.mybir as mybir
    import concourse.tile as tile
    from concourse.masks import make_identity

    nc = bacc.Bacc("TRN2", target_bir_lowering=False, debug=False)
    feats = nc.dram_tensor("features", [NUM_NODES, F], mybir.dt.bfloat16,
                           kind="ExternalInput")
    wt = nc.dram_tensor("wt", [2 * F, E], mybir.dt.bfloat16,
                        kind="ExternalInput")
    gidx = nc.dram_tensor("gidx", [B_LOCAL, SLOTS], mybir.dt.int32,
                          kind="ExternalInput")
    out = nc.dram_tensor("out", [E, B_LOCAL], mybir.dt.float32,
                         kind="ExternalOutput")

    with tile.TileContext(nc) as tc:
        with (
            tc.tile_pool(name="const", bufs=1) as constp,
            tc.tile_pool(name="gather", bufs=4) as gatherp,
            tc.tile_pool(name="red", bufs=2) as redp,
            tc.tile_pool(name="acc", bufs=3) as accp,
            tc.tile_pool(name="combT", bufs=2) as combp,
            tc.tile_pool(name="outs", bufs=3) as outsp,
            tc.tile_pool(name="psc", bufs=1, space="PSUM") as psc,
            tc.tile_pool(name="pts", bufs=2, space="PSUM") as ptsp,
            tc.tile_pool(name="ptn", bufs=2, space="PSUM") as ptnp,
            tc.tile_pool(name="psm", bufs=2, space="PSUM") as pmp,
        ):
            ident_f = constp.tile([P, P], mybir.dt.float32)
            make_identity(nc, ident_f[:])
            ident_b = constp.tile([P, P], mybir.dt.bfloat16)
            make_identity(nc, ident_b[:])
            scratch = psc.tile([P, P], mybir.dt.float32, tag="scratch")
            nc.tensor.transpose(out=scratch[:], in_=ident_f[:],
                                identity=ident_f[:])

            wtile = constp.tile([P, 4 * E], mybir.dt.bfloat16)
            nc.sync.dma_start(
                out=wtile[:].rearrange("k (c e) -> k c e", c=4),
                in_=wt.ap().rearrange("(c k) e -> k c e", k=P),
            )
            ixall = constp.tile([P, TILES * SLOTS], mybir.dt.int32)
            nc.sync.dma_start(
                out=ixall[:].rearrange("p (t s) -> p t s", t=TILES),
                in_=gidx.ap().rearrange("(t p) s -> p t s", p=P),
            )

            gts = []
            for t in range(TILES):
                g = gatherp.tile([P, SLOTS * F], mybir.dt.bfloat16,
                                 tag="g", name=f"g_{t}")
                for s in range(SLOTS):
                    nc.gpsimd.indirect_dma_start(
                        out=g[:, s * F:(s + 1) * F], out_offset=None,
                        in_=feats.ap()[:, :],
                        in_offset=bass.IndirectOffsetOnAxis(
                            ap=ixall[:, t * SLOTS + s:t * SLOTS + s + 1],
                            axis=0),
                    )
                gts.append(g)

            combT = None
            for t in range(TILES):
                g = gts[t]
                gv = g[:].rearrange("p (j f) -> p j f", j=SLOTS)
                # bf16 pairwise tree over slots 1..10 (contiguous APs)
                s1 = redp.tile([P, 5 * F], mybir.dt.bfloat16, tag="s1")
                s1v = s1[:].rearrange("p (j f) -> p j f", j=5)
                nc.vector.tensor_tensor(out=s1v, in0=gv[:, 1:6, :],
                                        in1=gv[:, 6:11, :],
                                        op=mybir.AluOpType.add)
                s2 = redp.tile([P, 2 * F], mybir.dt.bfloat16, tag="s2")
                s2v = s2[:].rearrange("p (j f) -> p j f", j=2)
                nc.vector.tensor_tensor(out=s2v, in0=s1v[:, 0:2, :],
                                        in1=s1v[:, 2:4, :],
                                        op=mybir.AluOpType.add)
                s3 = redp.tile([P, F], mybir.dt.bfloat16, tag="s3")
                nc.vector.tensor_tensor(out=s3[:], in0=s2v[:, 0, :],
                                        in1=s2v[:, 1, :],
                                        op=mybir.AluOpType.add)
                acc = accp.tile([P, F], mybir.dt.bfloat16, tag="acc",
                                name=f"acc_{t}")
                nc.vector.tensor_tensor(out=acc[:], in0=s3[:],
                                        in1=s1v[:, 4, :],
                                        op=mybir.AluOpType.add)

                ps = ptsp.tile([P, F], mybir.dt.bfloat16, tag="pts")
                nc.tensor.transpose(out=ps[:, 0:P], in_=g[:, 0:P],
                                    identity=ident_b[:])
                nc.tensor.transpose(out=ps[:, P:F], in_=g[:, P:F],
                                    identity=ident_b[:])
                pn = ptnp.tile([P, F], mybir.dt.bfloat16, tag="ptn")
                nc.tensor.transpose(out=pn[:, 0:P], in_=acc[:, 0:P],
                                    identity=ident_b[:])
                nc.tensor.transpose(out=pn[:, P:F], in_=acc[:, P:F],
                                    identity=ident_b[:])

                gq, bt = t // GROUP, t % GROUP
                if bt == 0:
                    combT = combp.tile([P, 4 * GROUP * P], mybir.dt.bfloat16,
                                       tag="combT", name=f"combT_{gq}")
                cv = combT[:].rearrange("p (k g b) -> p k g b", k=4, g=GROUP)
                nc.scalar.copy(out=cv[:, 0:2, bt, :],
                               in_=ps[:].rearrange("p (c b) -> p c b", c=2))
                nc.scalar.copy(out=cv[:, 2:4, bt, :],
                               in_=pn[:].rearrange("p (c b) -> p c b", c=2))

                if bt == GROUP - 1:
                    for eh in range(2):
                        pm = pmp.tile([P, GROUP * P], mybir.dt.float32,
                                      tag="pm")
                        for kc in range(4):
                            nc.tensor.matmul(
                                out=pm[:],
                                lhsT=wtile[:, kc * E + eh * P:
                                           kc * E + (eh + 1) * P],
                                rhs=combT[:, kc * GROUP * P:
                                          (kc + 1) * GROUP * P],
                                start=(kc == 0), stop=(kc == 3),
                            )
                        o = outsp.tile([P, GROUP * P], mybir.dt.float32,
                                       tag="o")
                        nc.scalar.activation(o[:], pm[:],
                                             mybir.ActivationFunctionType.Relu)
                        nc.sync.dma_start(
                            out=out.ap()[eh * P:(eh + 1) * P,
                                         gq * GROUP * P:(gq + 1) * GROUP * P],
                            in_=o[:])
    nc.compile()
    return nc


def _get_nc():
    if "nc" not in _cache:
        _cache["nc"] = _build()
    return _cache["nc"]


def _prep(features, W, nodes, neigh_idx):
    import ml_dtypes

    feats_b = np.ascontiguousarray(
        np.asarray(features, dtype=np.float32).astype(ml_dtypes.bfloat16))
    W = np.asarray(W, dtype=np.float32)
    nodes = np.asarray(nodes).astype(np.int32)
    neigh = np.asarray(neigh_idx).astype(np.int32)

    wt = np.ascontiguousarray(
        np.concatenate([W[:, :F].T, W[:, F:].T / NSAMP], axis=0)
    ).astype(ml_dtypes.bfloat16)

    in_maps = []
    for c in range(N_CORES):
        sl = slice(c * B_LOCAL, (c + 1) * B_LOCAL)
        gx = np.concatenate([nodes[sl, None], neigh[sl]], axis=1)
        in_maps.append({"features": feats_b, "wt": wt,
                        "gidx": np.ascontiguousarray(gx, dtype=np.int32)})
    return in_maps


def run(features, W, nodes, neigh_idx, trace=False):
    from concourse.bass_utils import run_bass_kernel_spmd

    in_maps = _prep(features, W, nodes, neigh_idx)
    res = run_bass_kernel_spmd(_get_nc(), in_maps,
                               core_ids=list(range(N_CORES)), trace=trace)
    out = np.concatenate([r["out"] for r in res.results], axis=1)
    return out, res


def kernel(features, W, nodes, neigh_idx):
    out, _ = run(features, W, nodes, neigh_idx)
    return out
